# revision 1
# baseline (speedup 1.0000x reference)
"""Trainium2 Bass kernel for nn_BaselineModel_74509092651544 (CLRS-style MPNN).

Strategy (v3)
-------------
Data-parallel over graphs: 32 graphs -> 8 cores x 4 graphs.  Messages run on
a padded CSR slot layout (only ~2.2k unique (src,dst) slots per graph survive
the masked max).  Versus the previous kernel:

  * m1[src]+m2[dst] gather-sums use fp8 DoubleRow matmuls (hi+res split of
    m12, exact to ~bf16) over an interleaved (gs,gd) one-hot operand: half
    the PE cycles of the two bf16 matmuls.
  * the bond term uses a single-fp8 [24,2]-packed DoubleRow matmul (2x).
  * masked max runs as one DVE segmented reduce per degree-segment (segments
    chosen by a DP over the receiver-degree profile, packed into one-bank
    512-col chunks).
  * LayerNorm per graph pair in feature-major via the GpSimd Q7 partition
    all-reduce (no PE transposes, no PSUM).
  * emission is a global slot-scheduled software pipeline: chunk stages are
    skewed (p1 lags 2 slots, p2 lags 6) so the in-order PE queue never waits
    on a fresh ACT/DVE result; consecutive layers overlap.
  * constraints honored: GPSIMD cannot read PSUM and supports no
    TensorScalarPtr, so the two relu passes live on ACT (wide chunks) and
    DVE (narrow chunks); reduces and residual casts on DVE.

All activation-dependent float math happens on device; the host precomputes
integer layout plus weight-only transforms (bond @ We, bias folds, fp8/bf16
weight splits).
"""

import sys
import numpy as np

sys.path.insert(0, "/opt/trn_rl_repo")

B, N, H, L, E, OUT = 32, 128, 128, 3, 65536, 128
M = 8                 # NeuronCores
BL = B // M           # graphs per core
EPS = 1e-5
AV, BV = 128, 16

_CACHE = {}


# --------------------------------------------------------------------------
# Host preprocessing: integer indexing / relayout + weight-only transforms.
# --------------------------------------------------------------------------

def _layout(deg):
    """Segment the (sorted-desc) receiver-degree profile and pack chunks.

    Returns (segs, chunks) where
      segs   = [(p0, R, K)]                 positions p0..p0+R padded to K
      chunks = [(w_pad, [(off, p0, R, K)])] slot-column chunks (<=512 wide,
                one PSUM bank each)
    """
    degS = -np.sort(-deg, axis=1)
    Kp = np.maximum(degS.max(axis=0), 1)          # [N], non-increasing
    C_SLOT, C_OP = 4.5, 120.0
    INF = 1e18
    dp = [INF] * (N + 1)
    dp[0] = 0.0
    prev = [-1] * (N + 1)
    for b in range(1, N + 1):
        for a in range(b):
            K = int(Kp[a])
            if (b - a) * K > 512:
                continue
            c = dp[a] + (b - a) * K * C_SLOT + C_OP
            if c < dp[b]:
                dp[b] = c
                prev[b] = a
    segs = []
    b = N
    while b > 0:
        a = prev[b]
        segs.append((a, b - a, int(Kp[a])))
        b = a
    segs.reverse()

    # pack segments into 512-wide chunks (first-fit decreasing)
    order = sorted(range(len(segs)), key=lambda i: -segs[i][1] * segs[i][2])
    chunks = []          # [ [size_used, [(off,p0,R,K)...]] ]
    for i in order:
        p0, R, K = segs[i]
        sz = R * K
        best = None
        for ci, ch in enumerate(chunks):
            if ch[0] + sz <= 512:
                if best is None or ch[0] > chunks[best][0]:
                    best = ci
        if best is None:
            chunks.append([sz, [(0, p0, R, K)]])
        else:
            ch = chunks[best]
            ch[1].append((ch[0], p0, R, K))
            ch[0] += sz
    out = []
    for used, lst in chunks:
        w = max(256, ((used + 7) // 8) * 8)
        assert w <= 512
        out.append((w, lst))
    # big chunks first (stable work profile)
    out.sort(key=lambda c: -c[0])
    return segs, out


def _prep(inputs):
    x = np.asarray(inputs["x"]).astype(np.int64)            # [B*N, 9]
    ea = np.asarray(inputs["edge_attr"]).astype(np.int64)   # [E, 3]
    ei = np.asarray(inputs["edge_index"]).astype(np.int64)  # [2, E]

    g = ei[0] // N
    s = ei[0] % N
    d = ei[1] % N
    key = (g * N + s) * N + d
    uniq, inv = np.unique(key, return_inverse=True)
    US = uniq.size
    ug = uniq // (N * N)
    us = (uniq // N) % N
    ud = uniq % N

    # bond one-hot counts per unique slot  [US, 48]
    oh48 = np.zeros((US, 48), np.float32)
    for c in range(3):
        np.add.at(oh48, (inv, ea[:, c] + 16 * c), 1.0)

    # unique in-degree per (graph, receiver)
    deg = np.zeros((B, N), np.int64)
    np.add.at(deg, (ug, ud), 1)
    assert (deg > 0).all(), "empty receivers unsupported in v3 layout"

    # receiver relabeling: position p holds the p-th highest-degree receiver
    rho = np.argsort(-deg, axis=1, kind="stable")        # [B, N] pos -> orig
    rho_inv = np.argsort(rho, axis=1)                    # orig -> pos

    segs, chunks = _layout(deg)
    S_graph = sum(w for (w, _) in chunks)
    S_core = BL * S_graph

    col_base_of_pos = np.zeros(N, np.int64)
    K_of_pos = np.zeros(N, np.int64)
    cb = 0
    for (w, lst) in chunks:
        for (off, p0, R, K) in lst:
            for r in range(R):
                col_base_of_pos[p0 + r] = cb + off + r * K
                K_of_pos[p0 + r] = K
        cb += w

    # slots ordered by (g, d, s): contiguous per receiver
    order = np.lexsort((us, ud, ug))
    og, od, osl = ug[order], ud[order], order
    osrc = us[order]
    recv_id = og * N + od
    first = np.concatenate([[0], np.flatnonzero(np.diff(recv_id)) + 1])
    k_rank = np.arange(len(og)) - first[np.searchsorted(recv_id[first], recv_id)]

    pos = rho_inv[og, od]
    core_r = og // BL
    col_r = (og % BL) * S_graph + col_base_of_pos[pos] + k_rank

    # padding: receivers with deg < K duplicate their first slot
    fg, fd = og[first], od[first]
    fpos = rho_inv[fg, fd]
    fdeg = deg[fg, fd]
    fK = K_of_pos[fpos]
    padc = (fK - fdeg).astype(np.int64)
    assert (padc >= 0).all()
    rep = np.repeat(np.arange(len(first)), padc)
    kpad = np.arange(len(rep)) - np.repeat(
        np.concatenate([[0], np.cumsum(padc)[:-1]]), padc
    ) + np.repeat(fdeg, padc)
    pg = fg[rep]
    core_p = pg // BL
    col_p = (pg % BL) * S_graph + col_base_of_pos[fpos[rep]] + kpad
    slot_p = osl[first][rep]
    src_p = osrc[first][rep]
    pos_p = fpos[rep]

    a_core = np.concatenate([core_r, core_p])
    a_col = np.concatenate([col_r, col_p])
    a_slot = np.concatenate([osl, slot_p])
    a_srcnew = np.concatenate([rho_inv[og, osrc], rho_inv[pg, src_p]])
    a_dstpos = np.concatenate([pos, pos_p])

    import ml_dtypes
    FP8 = ml_dtypes.float8_e4m3fn
    flat = a_core * S_core + a_col
    # interleaved (gs, gd) one-hot planes: gsd[node, plane, col]
    Gsd = np.zeros((M * S_core, 2, 128), np.float32)
    Gsd[flat, 0, a_srcnew] = 1.0
    Gsd[flat, 1, a_dstpos] = 1.0
    Gsd = np.ascontiguousarray(
        Gsd.reshape(M, S_core, 2, 128).transpose(0, 3, 2, 1)).astype(FP8)
    # soh24[p, i, col] = count of bond (p + 24*i)
    SOH = np.zeros((M * S_core, 48), np.float32)
    SOH[flat] = oh48[a_slot]
    assert float(SOH.max()) <= 240.0
    SOH = np.ascontiguousarray(
        SOH.reshape(M, S_core, 2, 24).transpose(0, 3, 2, 1)).astype(FP8)

    # atom one-hot per core: [9, 128, BL*N] in relabeled node order
    gg = np.repeat(np.arange(B), N)
    pp = np.tile(np.arange(N), B)
    orig = gg * N + rho[gg, pp]                    # [B*N] column -> orig node
    XOH = np.zeros((M, 9, AV, BL * N), np.float32)
    mcol = np.tile(np.arange(BL * N), M)
    mcore = np.repeat(np.arange(M), BL * N)
    for c in range(9):
        XOH[mcore, c, x[orig, c], mcol] = 1.0
    XOH = np.ascontiguousarray(XOH.transpose(0, 2, 1, 3)).astype(FP8)

    struct = dict(
        S_graph=S_graph, S_core=S_core,
        chunks=tuple((w, tuple(lst)) for (w, lst) in chunks),
    )
    percore = dict(Gsd=Gsd, SOH=SOH, XOH=XOH)
    return struct, percore


def _weight_arrays(inputs):
    import ml_dtypes
    f32 = np.float32
    FP8 = ml_dtypes.float8_e4m3fn
    BF16 = ml_dtypes.bfloat16
    A = {}

    Wm1 = np.asarray(inputs["Wm1"], f32)
    Wm2 = np.asarray(inputs["Wm2"], f32)
    cols = []
    wmap = {}

    def add(name, arr):
        wmap[name] = (sum(c.shape[1] for c in cols), arr.shape[1])
        cols.append(arr)

    # layer-0 block first so a partial DMA unblocks layer 0
    add("m12_0_0", np.concatenate([Wm1[0, 0:128], Wm2[0, 0:128]], 1))
    add("Wp1_0", np.asarray(inputs["Wp1"], f32)[0])
    add("Wp2_0", np.asarray(inputs["Wp2"], f32)[0])
    add("Wo1_0_0", np.asarray(inputs["Wo1"], f32)[0, 0:128])
    add("Wo2_0", np.asarray(inputs["Wo2"], f32)[0])
    add("Wh1", np.asarray(inputs["Wh1"], f32))
    add("Wh2", np.asarray(inputs["Wh2"], f32))
    for l in (1, 2):
        add(f"m12_{l}_0", np.concatenate([Wm1[l, 0:128], Wm2[l, 0:128]], 1))
        add(f"m12_{l}_1", np.concatenate([Wm1[l, 128:256], Wm2[l, 128:256]], 1))
        add(f"Wp1_{l}", np.asarray(inputs["Wp1"], f32)[l])
        add(f"Wp2_{l}", np.asarray(inputs["Wp2"], f32)[l])
        add(f"Wo1_{l}_0", np.asarray(inputs["Wo1"], f32)[l, 0:128])
        add(f"Wo1_{l}_1", np.asarray(inputs["Wo1"], f32)[l, 128:256])
        add(f"Wo2_{l}", np.asarray(inputs["Wo2"], f32)[l])
    A["wblob"] = np.ascontiguousarray(np.concatenate(cols, 1))
    A["_wmap"] = wmap

    # atom embedding, bf16 hi + bf16 residual  [AV, 2*9*H]
    atom = np.asarray(inputs["atom_emb"], f32)
    at = atom.transpose(1, 0, 2).reshape(AV, 9 * H)
    hi = at.astype(BF16)
    res = (at - hi.astype(f32)).astype(BF16)
    A["atomb"] = np.ascontiguousarray(np.concatenate([hi, res], 1))

    # bond weights: bw_l = bond48 @ We[l], packed [24, 2, H] single fp8
    bond48 = np.asarray(inputs["bond_emb"], f32).reshape(48, H)
    bws = []
    for l in range(L):
        bw = bond48 @ np.asarray(inputs["We"], f32)[l]          # [48, H]
        bws.append(bw.reshape(2, 24, H).transpose(1, 0, 2).reshape(24, 2 * H))
    A["bw24"] = np.ascontiguousarray(np.concatenate(bws, 1)).astype(FP8)

    # bias columns [H, 1 + 5*L + 2]
    nb = 1 + 5 * L + 2
    bc = np.zeros((H, nb), f32)
    bc[:, 0] = EPS
    for l in range(L):
        o = 1 + 5 * l
        bc[:, o + 0] = (np.asarray(inputs["bm1"], f32)[l]
                        + np.asarray(inputs["bm2"], f32)[l]
                        + np.asarray(inputs["be"], f32)[l]
                        + np.asarray(inputs["bg"], f32)[l])
        bc[:, o + 1] = np.asarray(inputs["bp1"], f32)[l]
        bc[:, o + 2] = (np.asarray(inputs["bo1"], f32)[l]
                        + np.asarray(inputs["bo2"], f32)[l]
                        + np.asarray(inputs["bp2"], f32)[l]
                        @ np.asarray(inputs["Wo2"], f32)[l])
        bc[:, o + 3] = np.asarray(inputs["ln_s"], f32)[l]
        bc[:, o + 4] = np.asarray(inputs["ln_b"], f32)[l]
    bc[:, 1 + 5 * L] = np.asarray(inputs["bh1"], f32)
    A["bias_cols"] = bc
    A["bh2_full"] = np.ascontiguousarray(
        np.asarray(inputs["bh2"], f32).reshape(OUT, 1))
    return A


# --------------------------------------------------------------------------
# Bass program.
# --------------------------------------------------------------------------

def _build_program(struct, wmap, wcols):
    import concourse.bacc as bacc
    import concourse.mybir as mybir
    import concourse.tile as tile

    F32 = mybir.dt.float32
    F32R = mybir.dt.float32r
    BF16 = mybir.dt.bfloat16
    FP8 = mybir.dt.float8e4

    S_core = struct["S_core"]

    nc = bacc.Bacc("TRN2", target_bir_lowering=False, debug=False)

    d = {}
    d["d_gsd"] = nc.dram_tensor("gsd", [128, 2, S_core], FP8, kind="ExternalInput")
    d["d_soh"] = nc.dram_tensor("soh", [24, 2, S_core], FP8, kind="ExternalInput")
    d["d_xoh"] = nc.dram_tensor("xoh", [AV, 9, BL * N], FP8, kind="ExternalInput")
    d["d_atomb"] = nc.dram_tensor("atomb", [AV, 2 * 9 * H], BF16, kind="ExternalInput")
    d["d_wblob"] = nc.dram_tensor("wblob", [128, wcols], F32R, kind="ExternalInput")
    d["d_bw24"] = nc.dram_tensor("bw24", [24, L * 2 * H], FP8, kind="ExternalInput")
    d["d_bc"] = nc.dram_tensor("bias_cols", [H, 1 + 5 * L + 2], F32, kind="ExternalInput")
    d["d_bh2"] = nc.dram_tensor("bh2_full", [OUT, 1], F32, kind="ExternalInput")
    d["d_out"] = nc.dram_tensor("out", [OUT, BL], F32, kind="ExternalOutput")

    with tile.TileContext(nc) as tc:
        _emit(tc, nc, d, struct, wmap, mybir)
    nc.compile()
    return nc


def _emit(tc, nc, d, struct, wmap, mybir):
    import contextlib
    from concourse import bass_isa
    ctx = contextlib.ExitStack()
    F32 = mybir.dt.float32
    F32R = mybir.dt.float32r
    BF16 = mybir.dt.bfloat16
    FP8 = mybir.dt.float8e4
    AF = mybir.ActivationFunctionType
    ALU = mybir.AluOpType
    AX = mybir.AxisListType
    PM = mybir.MatmulPerfMode

    S_graph = struct["S_graph"]
    S_core = struct["S_core"]
    chunks = struct["chunks"]
    NC = len(chunks)                      # chunks per graph
    JOBS = [(gg, ch) for gg in range(BL) for ch in range(NC)]
    NJ = len(JOBS)
    chunk_base = [0]
    for (w, _) in chunks:
        chunk_base.append(chunk_base[-1] + w)

    pG = ctx.enter_context(tc.tile_pool(name="pG", bufs=1))
    pW = ctx.enter_context(tc.tile_pool(name="pW", bufs=1))
    pM = ctx.enter_context(tc.tile_pool(name="pM", bufs=4))     # msgs tiles
    pS = ctx.enter_context(tc.tile_pool(name="pS", bufs=2))     # small tiles
    pNM = ctx.enter_context(tc.tile_pool(name="pNM", bufs=1))
    ps_pre = ctx.enter_context(tc.tile_pool(name="ps_pre", bufs=2, space="PSUM"))
    ps_p1 = ctx.enter_context(tc.tile_pool(name="ps_p1", bufs=3, space="PSUM"))
    ps_p2 = ctx.enter_context(tc.tile_pool(name="ps_p2", bufs=3, space="PSUM"))

    def mps(name, dt=F32, shape=(128, 512)):
        # misc matmul outputs share the p2 pool's rotation (PSUM is 8 banks)
        return ps_p2.tile(list(shape), dt, name=name, tag="p2",
                          padded_shape=[128, 2048 // mybir.dt.size(dt)])

    # ---- resident SBUF tensors
    gsd_sb = pG.tile([128, 2, S_core], FP8, name="gsd_sb")
    soh_sb = pG.tile([24, 2, S_core], FP8, name="soh_sb")
    wblob_sb = pW.tile([128, sum(w for (_, w) in wmap.values())], F32R,
                       name="wblob_sb")
    atomb_sb = pW.tile([AV, 2 * 9 * H], BF16, name="atomb_sb")
    xoh_sb = pW.tile([AV, 9, BL * N], FP8, name="xoh_sb")
    bw24_sb = pW.tile([24, L * 2 * H], FP8, name="bw24_sb")
    bc_sb = pW.tile([H, 1 + 5 * L + 2], F32, name="bc_sb")
    bh2_sb = pW.tile([OUT, 1], F32, name="bh2_sb")

    def W(name):
        off, w = wmap[name]
        return wblob_sb[:, off:off + w]

    ws1 = wmap["m12_1_0"][0]          # end of layer-0 + head block

    # ---- DMAs, in queue order: encoder inputs, layer-0 weights, graph data
    nc.sync.dma_start(atomb_sb[:, 0:9 * H], d["d_atomb"].ap()[:, 0:9 * H])
    nc.sync.dma_start(xoh_sb[:], d["d_xoh"].ap())
    ws0 = wmap["m12_0_0"][0] + wmap["m12_0_0"][1]
    nc.sync.dma_start(wblob_sb[:, 0:ws0], d["d_wblob"].ap()[:, 0:ws0])
    nc.sync.dma_start(atomb_sb[:, 9 * H:], d["d_atomb"].ap()[:, 9 * H:])
    w0 = chunks[0][0]
    slc0 = slice(0, w0)
    nc.sync.dma_start(gsd_sb[:, :, slc0], d["d_gsd"].ap()[:, :, slc0])
    nc.sync.dma_start(soh_sb[:, :, slc0], d["d_soh"].ap()[:, :, slc0])
    nc.sync.dma_start(wblob_sb[:, ws0:ws1], d["d_wblob"].ap()[:, ws0:ws1])
    sl0 = slice(w0, S_graph)
    nc.sync.dma_start(gsd_sb[:, :, sl0], d["d_gsd"].ap()[:, :, sl0])
    nc.sync.dma_start(soh_sb[:, :, sl0], d["d_soh"].ap()[:, :, sl0])
    nc.sync.dma_start(bc_sb[:], d["d_bc"].ap())
    nc.sync.dma_start(bw24_sb[:], d["d_bw24"].ap())
    for gg in range(1, BL):
        sl = slice(gg * S_graph, (gg + 1) * S_graph)
        nc.sync.dma_start(gsd_sb[:, :, sl], d["d_gsd"].ap()[:, :, sl])
        nc.sync.dma_start(soh_sb[:, :, sl], d["d_soh"].ap()[:, :, sl])
    nc.sync.dma_start(wblob_sb[:, ws1:], d["d_wblob"].ap()[:, ws1:])
    nc.sync.dma_start(bh2_sb[:], d["d_bh2"].ap())

    # ---- node features (feature-major)
    nf_ps = mps("nf_ps")
    for c in range(9):
        nc.tensor.matmul(nf_ps[:], atomb_sb[:, c * H:(c + 1) * H],
                         xoh_sb[:, c, :], start=(c == 0), stop=False)
    for c in range(9):
        nc.tensor.matmul(nf_ps[:], atomb_sb[:, (9 + c) * H:(10 + c) * H],
                         xoh_sb[:, c, :], start=False, stop=(c == 8))
    nf = pNM.tile([128, BL * N], F32R, name="nf")
    nc.scalar.activation(nf[:], nf_ps[:], AF.Copy)

    dr2 = lambda ap: ap.rearrange("p (two f) -> p two f", two=2)

    # Global slot-scheduled emission: layer l+1's first chunks overlap layer
    # l's LN drain.  Events registered per (slot); per-slot order = layer
    # registration order (earlier layer's chunk ops first).
    S1, S2 = 1, 5
    LAYER_SPAN = NJ + 4
    events = {}

    def at(slot, fn):
        events.setdefault(slot, []).append(fn)

    hid_of = [None] * L

    def schedule_layer(l, base_l):
        o = 1 + 5 * l
        bias_pre = bc_sb[:, o + 0:o + 1]
        bias_p1 = bc_sb[:, o + 1:o + 2]
        bias_h = bc_sb[:, o + 2:o + 3]
        ln_s = bc_sb[:, o + 3:o + 4]
        ln_b = bc_sb[:, o + 4:o + 5]
        bw_l = dr2(bw24_sb[:, l * 2 * H:(l + 1) * 2 * H])
        lay = {}                      # per-layer lazily-allocated tiles
        m12hi, m12res, m1_t, m2_t, h_fm, ln_st = {}, {}, {}, {}, {}, {}

        def emit_m12(gg):
            ps_m = mps("ps_m", shape=(128, 256))
            nc.tensor.matmul(ps_m[:], nf[:, gg * N:(gg + 1) * N],
                             W(f"m12_{l}_0"), start=True, stop=(l == 0))
            if l > 0:
                nc.tensor.matmul(ps_m[:], hid_of[l - 1][:, gg * N:(gg + 1) * N],
                                 W(f"m12_{l}_1"), start=False, stop=True)
            hi = pS.tile([128, 2 * H], FP8, name="m12hi", tag="m12hi")
            nc.scalar.activation(hi[:], ps_m[:], AF.Copy)
            res = pS.tile([128, 2 * H], FP8, name="m12res", tag="m12res")
            nc.vector.scalar_tensor_tensor(res[:], ps_m[:], 1.0, hi[:],
                                           op0=ALU.mult, op1=ALU.subtract)
            m12hi[gg], m12res[gg] = hi, res

        def emit_dr(gg, ch):
            w, _ = chunks[ch]
            cb = gg * S_graph + chunk_base[ch]
            pre = ps_pre.tile([128, 512], F32, name="pre", tag="pre")
            gsl = gsd_sb[:, :, cb:cb + w]
            ssl = soh_sb[:, :, cb:cb + w]
            nc.tensor.matmul(pre[:, 0:w], dr2(m12hi[gg][:]), gsl,
                             start=True, stop=False, perf_mode=PM.DoubleRow)
            nc.tensor.matmul(pre[:, 0:w], dr2(m12res[gg][:]), gsl,
                             start=False, stop=False, perf_mode=PM.DoubleRow)
            nc.tensor.matmul(pre[:, 0:w], bw_l, ssl,
                             start=False, stop=True, perf_mode=PM.DoubleRow)
            # msgs1 = relu(pre + bias_pre)   [ACT]
            m1s = pM.tile([128, 512], F32R, name="msgs1", tag="msgs1")
            nc.scalar.activation(m1s[:, 0:w], pre[:, 0:w], AF.Relu,
                                 bias=bias_pre)
            m1_t[(gg, ch)] = m1s

        def emit_p1(gg, ch):
            w, _ = chunks[ch]
            m1s = m1_t.pop((gg, ch))
            p1 = ps_p1.tile([128, 512], F32, name="p1", tag="p1")
            nc.tensor.matmul(p1[:, 0:w], W(f"Wp1_{l}"), m1s[:, 0:w],
                             start=True, stop=True)
            # msgs2 = relu(p1 + bp1)   [Pool for wide chunks, DVE for small]
            m2s = pM.tile([128, 512], F32R, name="msgs2", tag="msgs2")
            if w >= 420:
                nc.scalar.activation(m2s[:, 0:w], p1[:, 0:w], AF.Relu,
                                     bias=bias_p1)
            else:
                nc.vector.tensor_scalar(m2s[:, 0:w], p1[:, 0:w], bias_p1, 0.0,
                                        op0=ALU.add, op1=ALU.max)
            m2_t[(gg, ch)] = m2s

        def emit_p2(gg, ch):
            w, lst = chunks[ch]
            m2s = m2_t.pop((gg, ch))
            p2 = ps_p2.tile([128, 512], F32, name="p2", tag="p2")
            nc.tensor.matmul(p2[:, 0:w], W(f"Wp2_{l}"), m2s[:, 0:w],
                             start=True, stop=True)
            if "mmax" not in lay:
                lay["mmax"] = pM.tile([128, BL * N], F32R, name="msgs_max",
                                      tag="mmax", bufs=2)
            for (off, p0, R, K) in lst:
                nc.vector.tensor_reduce(
                    lay["mmax"][:, gg * N + p0:gg * N + p0 + R],
                    p2[:, off:off + R * K].rearrange("p (r k) -> p r k", r=R),
                    axis=AX.X, op=ALU.max)

        def emit_h(pair):
            g0 = 2 * pair
            sl = slice(g0 * N, (g0 + 2) * N)
            h_ps = mps("h_ps", shape=(128, 256))
            nc.tensor.matmul(h_ps[:], W(f"Wo1_{l}_0"), nf[:, sl],
                             start=True, stop=False)
            if l > 0:
                nc.tensor.matmul(h_ps[:], W(f"Wo1_{l}_1"),
                                 hid_of[l - 1][:, sl],
                                 start=False, stop=False)
            nc.tensor.matmul(h_ps[:], W(f"Wo2_{l}"), lay["mmax"][:, sl],
                             start=False, stop=True)
            hf = pS.tile([128, 256], F32R, name="h_fm", tag="h_fm")
            nc.vector.tensor_scalar(hf[:], h_ps[:], bias_h, 0.0,
                                    op0=ALU.add, op1=ALU.max)
            h_fm[pair] = hf

        def emit_ln(pair, step):
            st = ln_st.setdefault(pair, {})
            if step == 0:      # Pool: sum across partitions
                hf = h_fm[pair]
                s_bc = pS.tile([128, 256], F32, name="s_bc", tag="s_bc")
                nc.gpsimd.partition_all_reduce(s_bc[:], hf[:].bitcast(F32),
                                               channels=128,
                                               reduce_op=bass_isa.ReduceOp.add)
                st["s"] = s_bc
            elif step == 1:    # DVE: d = h - mean
                d = pS.tile([128, 256], F32R, name="d_ln", tag="d_ln")
                nc.vector.scalar_tensor_tensor(d[:], st.pop("s")[:], -1.0 / H,
                                               h_fm[pair][:].bitcast(F32),
                                               op0=ALU.mult, op1=ALU.add)
                st["d"] = d
            elif step == 2:    # ACT: d^2 ; Pool: sum d^2
                dsq = pS.tile([128, 256], F32R, name="dsq", tag="dsq")
                nc.gpsimd.tensor_tensor(dsq[:], st["d"][:].bitcast(F32),
                                        st["d"][:].bitcast(F32), op=ALU.mult)
                v_bc = pS.tile([128, 256], F32, name="v_bc", tag="v_bc")
                nc.gpsimd.partition_all_reduce(v_bc[:], dsq[:].bitcast(F32),
                                               channels=128,
                                               reduce_op=bass_isa.ReduceOp.add)
                st["v"] = v_bc
            elif step == 3:    # ACT: std = sqrt(v/H + eps)
                std = pS.tile([128, 256], F32, name="std", tag="std")
                nc.scalar.activation(std[:], st.pop("v")[:], AF.Sqrt,
                                     bias=bc_sb[:, 0:1], scale=1.0 / H)
                st["sd"] = std
            elif step == 4:    # DVE: rstd ; t = d * ln_s * rstd
                rstd = pS.tile([128, 256], F32, name="rstd", tag="rstd")
                nc.vector.reciprocal(rstd[:], st.pop("sd")[:])
                t = pS.tile([128, 256], F32R, name="t_ln", tag="t_ln")
                nc.vector.scalar_tensor_tensor(t[:], st.pop("d")[:], ln_s,
                                               rstd[:],
                                               op0=ALU.mult, op1=ALU.mult)
                st["t"] = t
            else:              # ACT: hid = t + ln_b (cast f32r)
                if hid_of[l] is None:
                    hid_of[l] = pNM.tile([128, BL * N], F32R,
                                         name=f"hid{l + 1}", tag="hid", bufs=2)
                g0 = 2 * pair
                nc.vector.tensor_scalar(hid_of[l][:, g0 * N:(g0 + 2) * N],
                                        st.pop("t")[:], ln_b, None,
                                        op0=ALU.add)
                ln_st.pop(pair)

        # register events; each graph's m12 prefetch must not be emitted
        # before the LN-final that writes its hid slice in the previous
        # layer (emission order = dependency-tracking order)
        for g in range(BL):
            want = base_l + (0 if (l == 0 and g == 0) else g * NC - 3)
            if l > 0:
                prev_final = (base_l - LAYER_SPAN) \
                    + (2 * (g // 2) + 2) * NC + S2 + 6
                want = max(want, prev_final + 1)
            at(want, (lambda gg=g: emit_m12(gg)))
        for j in range(NJ):
            at(base_l + j, (lambda jj=j: emit_dr(*JOBS[jj])))
            at(base_l + j + S1, (lambda jj=j: emit_p1(*JOBS[jj])))
            at(base_l + j + S2, (lambda jj=j: emit_p2(*JOBS[jj])))
        for pair in range(BL // 2):
            base = base_l + (2 * pair + 2) * NC + S2
            at(base, (lambda p=pair: emit_h(p)))
            for step in range(6):
                at(base + 1 + step, (lambda p=pair, s=step: emit_ln(p, s)))

    for l in range(L):
        schedule_layer(l, l * LAYER_SPAN)
    for s in sorted(events):
        for fn in events[s]:
            fn()

    # ---- pooling + prediction MLP
    ge_sum = pS.tile([128, BL], F32, name="ge_sum", tag="ge_sum")
    nc.vector.tensor_reduce(
        ge_sum[:], hid_of[L - 1][:].bitcast(F32)
        .rearrange("p (g n) -> p g n", g=BL),
        axis=AX.X, op=ALU.add)
    ge = pS.tile([128, BL], F32R, name="ge", tag="ge")
    nc.scalar.activation(ge[:], ge_sum[:], AF.Copy, scale=1.0 / N)
    o1 = mps("o1_ps", shape=(128, BL))
    nc.tensor.matmul(o1[:], W("Wh1"), ge[:], start=True, stop=True)
    t1 = pS.tile([128, BL], F32R, name="t1", tag="t1")
    nc.scalar.activation(t1[:], o1[:], AF.Relu,
                         bias=bc_sb[:, 1 + 5 * L:2 + 5 * L])
    o2 = mps("o2_ps", shape=(128, BL))
    nc.tensor.matmul(o2[:], W("Wh2"), t1[:], start=True, stop=True)
    out_sb = pS.tile([OUT, BL], F32, name="out_sb", tag="out_sb")
    nc.scalar.activation(out_sb[:], o2[:], AF.Identity, bias=bh2_sb[:])
    nc.sync.dma_start(d["d_out"].ap(), out_sb[:])
    ctx.close()


# --------------------------------------------------------------------------
# Entry point.
# --------------------------------------------------------------------------

def build(inputs):
    struct, percore = _prep(inputs)
    A = _weight_arrays(inputs)
    wmap = A.pop("_wmap")
    key = (struct["S_graph"], struct["chunks"])
    if key not in _CACHE:
        _CACHE[key] = _build_program(struct, wmap, A["wblob"].shape[1])
    nc = _CACHE[key]

    in_maps = []
    for c in range(M):
        im = dict(gsd=percore["Gsd"][c], soh=percore["SOH"][c],
                  xoh=percore["XOH"][c])
        for k, v in A.items():
            im[k] = v
        in_maps.append(im)
    return nc, in_maps, struct


def kernel(**inputs):
    from concourse import bass_utils
    nc, in_maps, struct = build(inputs)
    res = bass_utils.run_bass_kernel_spmd(nc, in_maps, core_ids=list(range(M)))
    out = np.zeros((B, OUT), np.float32)
    for c in range(M):
        out[c * BL:(c + 1) * BL] = res.results[c]["out"].T
    return out

